# revision 23
# baseline (speedup 1.0000x reference)
"""Trainium2 Bass kernel for nn_DYCEP_8572754723266.

CNN(3x stride-2 conv) -> fc -> 6x Mamba blocks -> head -> softmax-over-T.
Sharding: data-parallel over batch B=8, one batch element per NeuronCore.
"""

import numpy as np
import ml_dtypes
from contextlib import ExitStack

import concourse.bass as bass
import concourse.mybir as mybir
import concourse.tile as tile
from concourse import bacc
from concourse.bass_utils import run_bass_kernel_spmd

F32 = mybir.dt.float32
F32R = mybir.dt.float32r
BF16 = mybir.dt.bfloat16
AF = mybir.ActivationFunctionType
OP = mybir.AluOpType
AX = mybir.AxisListType

B, T, H, W = 8, 256, 64, 64
D_MODEL, N_LAYERS, D_STATE = 256, 6, 16
D_INNER = 2 * D_MODEL
DT_RANK = 16
D_CONV = 4
CNN_Z = 32
NES = 4  # d_inner slices of 128
NMD = 2  # d_model slices of 128

BF = ml_dtypes.bfloat16

# ---------------------------------------------------------------------------
# conv block tables
# ---------------------------------------------------------------------------


def _c1_blocks():
    # (lo, K): iy window per oy-block of 8; loaded into x-tile quadrant 32*b
    out = []
    for b in range(4):
        lo = max(0, 16 * b - 1)
        hi = min(63, 16 * b + 15)
        out.append((lo, hi - lo + 1))
    return out


def _c2_pieces():
    # z1 partition p = (iy % 8) * 16 + cin ; iyh = iy // 8 in free dim.
    # piece = ("full", iyh) K=128 base 0, or ("bnd", j) K=16 base 0 from z1b
    # (z1b row j holds iy = 8*j + 7).
    blocks = []
    for bp in range(4):
        pieces = []
        if bp > 0:
            pieces.append(("bnd", bp - 1))
        pieces.append(("full", bp))
        blocks.append(pieces)
    return blocks


def _c3_pieces():
    # z2 partition p = (iy % 4) * 32 + cin ; iyh = iy // 4.
    # piece = ("full", iyh) K=128 base 0, or ("bnd", iyh) K=32 base 96
    # (row iy = 4*iyh + 3 sits at partitions 96..127 of z2 block iyh).
    blocks = []
    for bp in range(4):
        pieces = []
        if bp > 0:
            pieces.append(("bnd", bp - 1))
        pieces.append(("full", bp))
        blocks.append(pieces)
    return blocks


C1B = _c1_blocks()
C2B = _c2_pieces()
C3B = _c3_pieces()


# padded coords: ix_pad = ix + 1, so matmul kx reads cols kx, kx+2, ... (full N)


# ---------------------------------------------------------------------------
# host-side weight preparation
# ---------------------------------------------------------------------------


def _host_prep(inp):
    d = {}
    f32 = np.float32

    w1 = np.asarray(inp["cnn_w1"], f32)
    w2 = np.asarray(inp["cnn_w2"], f32)
    w3 = np.asarray(inp["cnn_w3"], f32)

    # conv1 with kx folded into K: rows (kx, iy-rel), cols (block, oyl, cout)
    c1w = np.zeros((51, 4 * 128), f32)
    for b, (lo, K) in enumerate(C1B):
        for kx in range(3):
            for oyl in range(8):
                oy = 8 * b + oyl
                for cout in range(16):
                    m = oyl * 16 + cout
                    for r in range(K):
                        ky = (lo + r) - 2 * oy + 1
                        if 0 <= ky <= 2:
                            c1w[kx * K + r, b * 128 + m] = w1[cout, 0, ky, kx]
    d["c1w"] = c1w.astype(BF)

    n2 = sum(len(p) for p in C2B)
    c2w = np.zeros((128, n2 * 3 * 128), f32)
    idx = 0
    for bp, pieces in enumerate(C2B):
        for (kind, j) in pieces:
            rows = range(8 * j, 8 * j + 8) if kind == "full" else [8 * j + 7]
            for kx in range(3):
                col0 = idx * 128
                idx += 1
                for oyl in range(4):
                    oy = 4 * bp + oyl
                    for cout in range(32):
                        m = oyl * 32 + cout
                        for rr, iy in enumerate(rows):
                            ky = iy - 2 * oy + 1
                            if 0 <= ky <= 2:
                                c2w[rr * 16 : rr * 16 + 16, col0 + m] = w2[cout, :, ky, kx]
    d["c2w"] = c2w.astype(BF)

    n3 = sum(len(p) for p in C3B)
    c3w = np.zeros((128, n3 * 3 * 64), f32)
    idx = 0
    for bp, pieces in enumerate(C3B):
        for (kind, j) in pieces:
            if kind == "full":
                rows = [(rr, 4 * j + rr) for rr in range(4)]  # (slab row grp, iy)
                rbase = 0
            else:
                rows = [(0, 4 * j + 3)]
                rbase = 0
            for kx in range(3):
                col0 = idx * 64
                idx += 1
                for oyl in range(2):
                    oy = 2 * bp + oyl
                    for cout in range(32):
                        m = oyl * 32 + cout
                        for rr, iy in rows:
                            ky = iy - 2 * oy + 1
                            if 0 <= ky <= 2:
                                c3w[rbase + rr * 32 : rbase + rr * 32 + 32, col0 + m] = w3[
                                    cout, :, ky, kx
                                ]
    d["c3w"] = c3w.astype(BF)

    d["c1b"] = np.tile(np.asarray(inp["cnn_b1"], f32), 8).reshape(128, 1)
    d["c2b"] = np.tile(np.asarray(inp["cnn_b2"], f32), 4).reshape(128, 1)
    d["c3b"] = np.tile(np.asarray(inp["cnn_b3"], f32), 2).reshape(64, 1)

    fcw = np.asarray(inp["fc_w"], f32) / 64.0  # pool-mean folded
    d["fcw"] = np.ascontiguousarray(fcw.T).astype(BF)  # [32, 256]
    d["fcb"] = np.ascontiguousarray(
        np.asarray(inp["fc_b"], f32).reshape(NMD, 128).T
    )  # [128, 2]

    d["ones"] = np.ones((128, 1), f32).astype(BF)
    d["onesrow"] = np.ones((1, 256), f32).astype(BF)

    nw = np.asarray(inp["norm_w"], f32)
    ipw = np.asarray(inp["in_proj_w"], f32)
    xpw = np.asarray(inp["x_proj_w"], f32)
    dpw = np.asarray(inp["dt_proj_w"], f32)
    opw = np.asarray(inp["out_proj_w"], f32)
    cdw = np.asarray(inp["conv1d_w"], f32)
    cdb = np.asarray(inp["conv1d_b"], f32)
    dpb = np.asarray(inp["dt_proj_b"], f32)
    Dp = np.asarray(inp["Dp"], f32)

    # u2 = 2*silu(u-path), zsil2 = 2*silu(z): fold 0.5s into downstream weights.
    wbf = np.zeros((N_LAYERS, 128, 2048 + 1024 + 256 + 2048), f32)
    for l in range(N_LAYERS):
        wtl = (ipw[l] * nw[l][None, :]).T  # (256, 1024)
        # x-half as lhsT tiles (es 0..3), z-half as rhs [d, 512] per kd
        for kd in range(2):
            for es in range(NES):
                wbf[l, :, (kd * 4 + es) * 128 : (kd * 4 + es + 1) * 128] = wtl[
                    kd * 128 : (kd + 1) * 128, es * 128 : (es + 1) * 128
                ]
            wbf[l, :, 1024 + kd * 512 : 1024 + (kd + 1) * 512] = wtl[
                kd * 128 : (kd + 1) * 128, 512:1024
            ]
        otl = 0.5 * opw[l].T  # (512, 256); 0.5 from zsil2
        for es in range(NES):
            for md in range(NMD):
                wbf[l, :, 2048 + (es * NMD + md) * 128 : 2048 + (es * NMD + md + 1) * 128] = otl[
                    es * 128 : (es + 1) * 128, md * 128 : (md + 1) * 128
                ]
        # x_proj: 0.5 from u2; B rows get another 0.5 (du = delta*u2 = 2*delta*u)
        # output rows padded to 64: dt 0:16, pad, B 32:48, C 48:64
        xtl = 0.5 * xpw[l].T.copy()  # (512, 48)
        xtl[:, 16:32] *= 0.5
        xtl64 = np.zeros((512, 64), f32)
        xtl64[:, 0:16] = xtl[:, 0:16]
        xtl64[:, 32:64] = xtl[:, 16:48]
        for es in range(NES):
            wbf[l, :, 3072 + es * 64 : 3072 + (es + 1) * 64] = xtl64[
                es * 128 : (es + 1) * 128
            ]
        # depthwise conv1d as diagonal matmuls: lhsT = diag(w[es-slice, k])
        for es in range(NES):
            for k in range(4):
                col0 = 3328 + (es * 4 + k) * 128
                wbf[l, np.arange(128), col0 + np.arange(128)] = cdw[
                    l, es * 128 : (es + 1) * 128, k
                ]
    d["wbf"] = wbf.astype(BF)

    # dt_proj as rhs [17, 512]: rows 0:16 = dpw.T, row 16 = dpb (bias via
    # the augmented ones row of bcq2)
    dpt = np.empty((N_LAYERS, 17, 512), f32)
    for l in range(N_LAYERS):
        dpt[l, 0:16] = dpw[l].T
        dpt[l, 16] = dpb[l]
    d["dpw"] = dpt.astype(BF)

    # SSM helper constants
    tt_i = np.arange(128)
    d["tri"] = (tt_i[:, None] <= tt_i[None, :]).astype(BF)  # [t, t'] inclusive
    d["eye"] = np.eye(128, dtype=np.float32).astype(BF)
    eyeb = np.zeros((64, 32), np.float32)
    eyeb[32:64] = np.eye(32)
    d["eyeb"] = eyeb.astype(BF)
    sel = np.zeros((16, 16 * 128), f32)
    for n in range(16):
        sel[n, n * 128 : (n + 1) * 128] = 1.0
    d["sel"] = sel.astype(BF)
    d["nneg"] = -np.arange(1.0, 17.0, dtype=f32).reshape(16, 1)
    selc = np.zeros((128, 256), f32)
    for n in range(16):
        selc[:, n * 16 + n] = 1.0
    d["selc"] = selc.astype(BF)
    d["drow"] = (0.5 * Dp).astype(BF)  # [N_LAYERS, 512]

    # f32 pack: cdb (4) | cdb/2 (4) | dpb (4) | 0.5*Dp (4)
    wf = np.zeros((N_LAYERS, 128, 16), f32)
    wf[:, :, 0:4] = cdb.reshape(N_LAYERS, NES, 128).transpose(0, 2, 1)
    wf[:, :, 4:8] = 0.5 * cdb.reshape(N_LAYERS, NES, 128).transpose(0, 2, 1)
    wf[:, :, 8:12] = dpb.reshape(N_LAYERS, NES, 128).transpose(0, 2, 1)
    wf[:, :, 12:16] = 0.5 * Dp.reshape(N_LAYERS, NES, 128).transpose(0, 2, 1)
    d["wf32"] = wf

    nfw = np.asarray(inp["norm_f_w"], f32)
    hw1 = np.asarray(inp["head_w1"], f32) * nfw[None, :]
    hw1t = hw1.T  # (256, 64)
    d["hw1"] = np.concatenate([hw1t[0:128], hw1t[128:256]], axis=1).astype(BF)
    d["hb1"] = np.asarray(inp["head_b1"], f32).reshape(64, 1)
    d["hw2"] = np.ascontiguousarray(0.5 * np.asarray(inp["head_w2"], f32).T).astype(BF)
    d["hb2"] = np.asarray(inp["head_b2"], f32).reshape(1, 1)
    return d


WSPECS = [
    ("c1w", (51, 4 * 128), BF16),
    ("c2w", (128, sum(len(p) for p in C2B) * 3 * 128), BF16),
    ("c3w", (128, sum(len(p) for p in C3B) * 3 * 64), BF16),
    ("c1b", (128, 1), F32),
    ("c2b", (128, 1), F32),
    ("c3b", (64, 1), F32),
    ("fcw", (32, 256), BF16),
    ("fcb", (128, 2), F32),
    ("ones", (128, 1), BF16),
    ("onesrow", (1, 256), BF16),
    ("wbf", (N_LAYERS, 128, 2048 + 1024 + 256 + 2048), BF16),
    ("dpw", (N_LAYERS, 17, 512), BF16),
    ("wf32", (N_LAYERS, 128, 16), F32),
    ("hw1", (128, 128), BF16),
    ("hb1", (64, 1), F32),
    ("hw2", (64, 1), BF16),
    ("hb2", (1, 1), F32),
    ("tri", (128, 128), BF16),
    ("eye", (128, 128), BF16),
    ("eyeb", (64, 32), BF16),
    ("sel", (16, 16 * 128), BF16),
    ("selc", (128, 256), BF16),
    ("nneg", (16, 1), F32),
    ("drow", (N_LAYERS, 512), BF16),
]


# ---------------------------------------------------------------------------
# device program
# ---------------------------------------------------------------------------


def _emit(ctx: ExitStack, tc, ins, out_ap):
    nc = tc.nc
    x = ins["x"]

    wsb = ctx.enter_context(tc.tile_pool(name="wsb", bufs=1))
    wt = {}
    for name in ("c1w", "c2w", "c3w", "c1b", "c2b", "c3b", "fcw", "fcb", "ones",
                 "hw1", "hb1", "hw2", "hb2", "tri", "eye", "eyeb", "sel",
                 "selc", "nneg"):
        ap = ins[name]
        t = wsb.tile(list(ap.shape), ap.dtype, tag=name)
        nc.sync.dma_start(out=t[:], in_=ap[:])
        wt[name] = t

    hp = ctx.enter_context(tc.tile_pool(name="hres", bufs=1))
    hresC = hp.tile([128, 2, 256], F32, tag="hresC")
    zpp = ctx.enter_context(tc.tile_pool(name="zpp", bufs=1))

    # ---------------- CNN ----------------
    with ExitStack() as cnx:
        xp = cnx.enter_context(tc.tile_pool(name="xp", bufs=3))
        z1p = cnx.enter_context(tc.tile_pool(name="z1p", bufs=2))
        z2p = cnx.enter_context(tc.tile_pool(name="z2p", bufs=2))
        z3p = cnx.enter_context(tc.tile_pool(name="z3p", bufs=2))
        cp1 = cnx.enter_context(tc.tile_pool(name="cp1", bufs=4, space="PSUM"))
        cp2 = cnx.enter_context(tc.tile_pool(name="cp2", bufs=2, space="PSUM"))
        cp3 = cnx.enter_context(tc.tile_pool(name="cp3", bufs=2, space="PSUM"))

        zp = zpp.tile([64, 256], F32)

        for c64 in range(4):
            z3 = z3p.tile([64, 64, 4, 8], BF16)  # (f64, oyh, ox)
            z2 = z2p.tile([128, 2, 32, 4, 18], BF16)  # (c32, f32, iyh, ixpad)
            z2b = z2p.tile([32, 2, 32, 3, 18], BF16, tag="z2b")  # bnd rows
            nc.vector.memset(z2[:, :, :, :, 0:1], 0.0)
            nc.vector.memset(z2[:, :, :, :, 17:18], 0.0)
            for c32 in range(2):
                z1 = z1p.tile([128, 2, 16, 4, 34], BF16)  # (c16, f16, iyh, ixpad)
                nc.vector.memset(z1[:, :, :, :, 0:1], 0.0)
                nc.vector.memset(z1[:, :, :, :, 33:34], 0.0)
                z1b = z1p.tile([16, 2, 16, 3, 34], BF16, tag="z1b")  # bnd rows
                for c16 in range(2):
                    f0 = (c64 * 4 + c32 * 2 + c16) * 16
                    for b, (lo, K) in enumerate(C1B):
                        xk = xp.tile([51, 16, 32], BF16, tag=f"xk{b}")
                        for kx in range(3):
                            nc.sync.dma_start(
                                out=xk[kx * K : (kx + 1) * K],
                                in_=x[kx, lo : lo + K, f0 : f0 + 16, :],
                            )
                        ps = cp1.tile([128, 16, 32], F32)
                        nc.tensor.matmul(
                            ps[:],
                            wt["c1w"][0 : 3 * K, b * 128 : (b + 1) * 128],
                            xk[0 : 3 * K],
                            start=True,
                            stop=True,
                        )
                        nc.scalar.activation(
                            z1[:, c16, :, b, 1:33], ps[:], AF.Relu, bias=wt["c1b"][:]
                        )
                        if b < 3:
                            nc.gpsimd.dma_start(
                                out=z1b[:, c16, :, b, :],
                                in_=z1[112:128, c16, :, b, :],
                            )
                # conv2 over the 32-frame chunk
                for bp, pieces in enumerate(C2B):
                    ps = cp2.tile([128, 32, 16], F32)
                    nmm = len(pieces) * 3
                    im = 0
                    for pi, (kind, j) in enumerate(pieces):
                        pidx = sum(len(p) for p in C2B[:bp]) + pi
                        for kx in range(3):
                            if kind == "full":
                                rhs = z1[:, :, :, j, kx : kx + 31 : 2]
                                K = 128
                            else:
                                rhs = z1b[:, :, :, j, kx : kx + 31 : 2]
                                K = 16
                            lhs = wt["c2w"][
                                0:K,
                                (pidx * 3 + kx) * 128 : (pidx * 3 + kx + 1) * 128,
                            ]
                            im += 1
                            nc.tensor.matmul(
                                ps[:],
                                lhs,
                                rhs,
                                start=(im == 1),
                                stop=(im == nmm),
                            )
                    nc.vector.tensor_scalar(
                        z2[:, c32, :, bp, 1:17], ps[:], wt["c2b"][:], 0.0,
                        OP.add, OP.max,
                    )
                    if bp < 3:
                        nc.gpsimd.dma_start(
                            out=z2b[:, c32, :, bp, :],
                            in_=z2[96:128, c32, :, bp, :],
                        )
            # conv3 over the 64-frame chunk
            for bp, pieces in enumerate(C3B):
                ps = cp3.tile([64, 64, 8], F32)
                nmm = len(pieces) * 3
                im = 0
                for pi, (kind, j) in enumerate(pieces):
                    pidx = sum(len(p) for p in C3B[:bp]) + pi
                    for kx in range(3):
                        if kind == "full":
                            rhs = z2[:, :, :, j, kx : kx + 15 : 2]
                            lhs = wt["c3w"][
                                0:128,
                                (pidx * 3 + kx) * 64 : (pidx * 3 + kx + 1) * 64,
                            ]
                        else:
                            rhs = z2b[:, :, :, j, kx : kx + 15 : 2]
                            lhs = wt["c3w"][
                                0:32,
                                (pidx * 3 + kx) * 64 : (pidx * 3 + kx + 1) * 64,
                            ]
                        im += 1
                        nc.tensor.matmul(
                            ps[:],
                            lhs,
                            rhs,
                            start=(im == 1),
                            stop=(im == nmm),
                        )
                nc.scalar.activation(
                    z3[:, :, bp, :], ps[:], AF.Relu, bias=wt["c3b"][:]
                )
            # spatial mean (x 1/64 folded into fcw): sum over (oyh, ox)
            nc.vector.tensor_reduce(
                zp[:, c64 * 64 : (c64 + 1) * 64], z3[:], AX.XY, OP.add
            )

        # fold (oyl 2) partition pairs: zq = zp[0:32] + zp[32:64]
        zq = zpp.tile([32, 256], F32, tag="zq")
        nc.sync.dma_start(out=zq[:], in_=zp[32:64, :])
        zfold = zpp.tile([32, 256], BF16, tag="zfold")
        nc.vector.tensor_tensor(zfold[:], zp[0:32, :], zq[:], OP.add)

    # ---------------- fc (CNN pools closed; use mamba psum pool) ----------------
    lwp = ctx.enter_context(tc.tile_pool(name="lwp", bufs=2))
    mps = ctx.enter_context(tc.tile_pool(name="mps", bufs=2, space="PSUM"))
    sps = ctx.enter_context(tc.tile_pool(name="sps", bufs=1, space="PSUM"))
    bps = ctx.enter_context(tc.tile_pool(name="bps", bufs=1, space="PSUM"))
    tps = ctx.enter_context(tc.tile_pool(name="tps", bufs=1, space="PSUM"))
    qps = ctx.enter_context(tc.tile_pool(name="qps", bufs=2, space="PSUM"))
    lcl = ctx.enter_context(tc.tile_pool(name="lcl", bufs=1))
    lc2 = ctx.enter_context(tc.tile_pool(name="lc2", bufs=2))
    epp = ctx.enter_context(tc.tile_pool(name="epp", bufs=3))
    zp2 = ctx.enter_context(tc.tile_pool(name="zp2", bufs=3))

    for md in range(NMD):
        ps = mps.tile([128, 256], F32, tag="mm")
        nc.tensor.matmul(
            ps[:], wt["fcw"][:, md * 128 : (md + 1) * 128], zfold[:],
            start=True, stop=True,
        )
        nc.scalar.activation(
            hresC[:, md, :], ps[:], AF.Identity, bias=wt["fcb"][:, md : md + 1]
        )

    # ---------------- Mamba layers (time-major SSM) ----------------
    for l in range(N_LAYERS):
        wb = lwp.tile([128, 5376], BF16, tag="wb")
        nc.gpsimd.dma_start(out=wb[:], in_=ins["wbf"][l])
        dpw_t = lwp.tile([17, 512], BF16, tag="dpw")
        nc.gpsimd.dma_start(out=dpw_t[:], in_=ins["dpw"][l])
        wf = lwp.tile([128, 16], F32, tag="wf")
        nc.gpsimd.dma_start(out=wf[:], in_=ins["wf32"][l])
        drow = lwp.tile([1, 512], BF16, tag="drow")
        nc.gpsimd.dma_start(out=drow[:], in_=ins["drow"][l : l + 1, :])
        dbc_t = lwp.tile([128, 512], BF16, tag="dbc")
        nc.gpsimd.partition_broadcast(dbc_t[:], drow[0:1, :])

        # --- rmsnorm (norm_w folded into in_proj weights) ---
        sqC = lcl.tile([128, 2, 256], BF16, tag="sqC")
        for md in range(NMD):
            nc.scalar.activation(sqC[:, md, :], hresC[:, md, :], AF.Square)
        shr = sps.tile([64, 256], F32, tag="shr")
        ssps = shr[0:1, :]
        for md in range(NMD):
            nc.tensor.matmul(
                ssps, wt["ones"][:], sqC[:, md, :],
                start=(md == 0), stop=(md == NMD - 1),
            )
        eps1 = lcl.tile([1, 1], F32, tag="eps1")
        nc.vector.memset(eps1[:], 1e-5)
        sv = lcl.tile([1, 256], F32, tag="sv")
        nc.scalar.activation(sv[:], ssps, AF.Sqrt, scale=1.0 / 256.0, bias=eps1[:])
        rstd = lcl.tile([1, 256], F32, tag="rstd")
        nc.vector.reciprocal_approx_fast(rstd[:], sv[:])
        rb = lcl.tile([128, 256], F32, tag="rb")
        nc.gpsimd.partition_broadcast(rb[:], rstd[0:1, :])
        hnC = lcl.tile([128, 2, 256], BF16, tag="hnC")
        for md in range(NMD):
            nc.vector.tensor_tensor(hnC[:, md, :], hresC[:, md, :], rb[:], OP.mult)

        # --- in_proj x-half (feature-major, es 0..3) ---
        xinC = lcl.tile([128, 4, 260], BF16, tag="xinC")
        nc.vector.memset(xinC[:, :, 0:3], 0.0)
        for es in range(NES):
            ps = mps.tile([128, 256], F32, tag="mm")
            for kd in range(2):
                nc.tensor.matmul(
                    ps[:],
                    wb[:, (kd * 4 + es) * 128 : (kd * 4 + es + 1) * 128],
                    hnC[:, kd, :],
                    start=(kd == 0),
                    stop=(kd == 1),
                )
            nc.scalar.activation(xinC[:, es, 3:259], ps[:], AF.Copy)

        # --- in_proj z-half (time-major) + silu ---
        zsil2T = lcl.tile([128, 2, 512], BF16, tag="zsil2T")
        for tcc in range(2):
            psz = bps.tile([128, 512], F32, tag="big")
            for kd in range(2):
                nc.tensor.matmul(
                    psz[:],
                    hnC[:, kd, tcc * 128 : (tcc + 1) * 128],
                    wb[:, 1024 + kd * 512 : 1024 + (kd + 1) * 512],
                    start=(kd == 0),
                    stop=(kd == 1),
                )
            thz = lc2.tile([128, 512], BF16, tag="thz")
            zc = lc2.tile([128, 512], BF16, tag="zc")
            nc.scalar.activation(thz[:], psz[:], AF.Tanh, scale=0.5)
            nc.scalar.activation(zc[:], psz[:], AF.Copy)
            nc.vector.scalar_tensor_tensor(
                zsil2T[:, tcc, :], thz[:], 1.0, zc[:], OP.add, OP.mult
            )

        # --- depthwise conv1d as diagonal PE matmuls (feature-major) ---
        xcC = lcl.tile([128, 4, 256], BF16, tag="xcC")
        thuC = lcl.tile([128, 4, 256], BF16, tag="thuC")
        for es in range(NES):
            pc = mps.tile([128, 256], F32, tag="mm")
            for k in range(4):
                nc.tensor.matmul(
                    pc[:],
                    wb[:, 3328 + (es * 4 + k) * 128 : 3328 + (es * 4 + k + 1) * 128],
                    xinC[:, es, k : k + 256],
                    start=(k == 0),
                    stop=(k == 3),
                )
            nc.scalar.activation(
                xcC[:, es, :], pc[:], AF.Identity, bias=wf[:, 0 + es : 1 + es]
            )
            nc.scalar.activation(
                thuC[:, es, :], pc[:], AF.Tanh, scale=0.5,
                bias=wf[:, 4 + es : 5 + es],
            )
        u2C = lcl.tile([128, 4, 256], BF16, tag="u2C")
        nc.vector.scalar_tensor_tensor(
            u2C[:].rearrange("p a t -> p (a t)"),
            thuC[:].rearrange("p a t -> p (a t)"),
            1.0,
            xcC[:].rearrange("p a t -> p (a t)"),
            OP.add,
            OP.mult,
        )

        # --- x_proj -> (dt, B, C); augmented with a ones row for dt bias ---
        shr2 = sps.tile([64, 256], F32, tag="shr")
        dbc = shr2[:]
        for es in range(NES):
            nc.tensor.matmul(
                dbc, wb[:, 3072 + es * 64 : 3072 + (es + 1) * 64], u2C[:, es, :],
                start=(es == 0), stop=(es == NES - 1),
            )
        bcq2 = lcl.tile([64, 256], BF16, tag="bcq2")
        nc.scalar.activation(bcq2[0:16, :], dbc[0:16, :], AF.Copy)
        nc.gpsimd.dma_start(out=bcq2[16:17, :], in_=ins["onesrow"][:])
        nc.scalar.activation(bcq2[32:64, :], dbc[32:64, :], AF.Copy)

        # --- B/C transpose to time-major per-partition scalars ---
        BCt = lcl.tile([128, 2, 32], F32, tag="BCt")
        for tcc in range(2):
            pstf = tps.tile([128, 512], BF16, tag="tpb")
            pst = pstf[:, 0:32]
            nc.tensor.transpose(
                pst, bcq2[32:64, tcc * 128 : (tcc + 1) * 128],
                wt["eyeb"][32:64, :],
            )
            nc.scalar.activation(BCt[:, tcc, :], pst, AF.Copy)

        # --- dt_proj (time-major) -> delta; softplus via 2-term series ---
        deltaT = lcl.tile([128, 2, 512], BF16, tag="deltaT")
        for tcc in range(2):
            psd = bps.tile([128, 512], F32, tag="big")
            nc.tensor.matmul(
                psd[:], bcq2[0:17, tcc * 128 : (tcc + 1) * 128], dpw_t[:],
                start=True, stop=True,
            )
            spe = lc2.tile([128, 512], F32, tag="spe")
            nc.scalar.activation(spe[:], psd[:], AF.Exp)
            sp1 = lc2.tile([128, 512], F32, tag="sp1")
            nc.vector.tensor_scalar(sp1[:], spe[:], -0.5, 1.0, OP.mult, OP.add)
            nc.vector.tensor_tensor(deltaT[:, tcc, :], spe[:], sp1[:], OP.mult)

        # --- u2 transpose (PE) -> u2T; du = delta * u2 ---
        u2T = lcl.tile([128, 2, 512], BF16, tag="u2T")
        duT = lcl.tile([128, 2, 512], BF16, tag="duT")
        for tcc in range(2):
            pst = tps.tile([128, 512], BF16, tag="tpb")
            for es in range(NES):
                nc.tensor.transpose(
                    pst[:, es * 128 : (es + 1) * 128],
                    u2C[:, es, tcc * 128 : (tcc + 1) * 128],
                    wt["eye"][:],
                )
            nc.scalar.activation(u2T[:, tcc, :], pst[:], AF.Copy)
            nc.vector.tensor_tensor(
                duT[:, tcc, :], deltaT[:, tcc, :], u2T[:, tcc, :], OP.mult
            )

        # --- S = inclusive cumsum of delta over t (PE triangular matmul) ---
        S_sb = lcl.tile([128, 2, 512], F32, tag="S_sb")
        for tcc in range(2):
            psS = bps.tile([128, 512], F32, tag="big")
            nc.tensor.matmul(
                psS[:], wt["tri"][:], deltaT[:, tcc, :], start=True, stop=True
            )
            nc.scalar.activation(S_sb[:, tcc, :], psS[:], AF.Copy)

        # --- SSM: y[t,e] = sum_n C_n[t] * en_n * cumsum_t(du*B_n*ep_n) ---
        hend_raw = lcl.tile([16, 512], F32, tag="hend_raw")
        hend = lcl.tile([16, 512], BF16, tag="hend")
        y3T = lcl.tile([128, 2, 512], BF16, tag="y3T")
        for tcc in range(2):
            accs = []
            if tcc == 0:
                psend = sps.tile([16, 512], F32, tag="hendp")
            for n in range(D_STATE):
                a = float(n + 1)
                ep = epp.tile([128, 512], BF16, tag="ep")
                nc.scalar.activation(ep[:], S_sb[:, tcc, :], AF.Exp, scale=a)
                en = epp.tile([128, 512], BF16, tag="en")
                nc.scalar.activation(en[:], S_sb[:, tcc, :], AF.Exp, scale=-a)
                duB = epp.tile([128, 512], BF16, tag="duB")
                nc.gpsimd.tensor_scalar(
                    duB[:], duT[:, tcc, :], BCt[:, tcc, n : n + 1], None, OP.mult
                )
                q = epp.tile([128, 512], BF16, tag="q")
                nc.vector.tensor_tensor(q[:], duB[:], ep[:], OP.mult)
                psq = qps.tile([128, 512], F32, tag="qs")
                nc.tensor.matmul(
                    psq[:], wt["tri"][:], q[:], start=True, stop=(tcc == 0)
                )
                if tcc == 1:
                    nc.tensor.matmul(
                        psq[:], wt["sel"][:, n * 128 : (n + 1) * 128], hend[:],
                        start=False, stop=True,
                    )
                if tcc == 0:
                    # route sum_t q_n[t,e] to partition n of psend
                    nc.tensor.matmul(
                        psend[:], wt["selc"][:, n * 16 : (n + 1) * 16], q[:],
                        start=(n == 0), stop=(n == D_STATE - 1),
                    )
                z = zp2.tile([128, 512], BF16, tag=f"z{n % 4}")
                nc.vector.scalar_tensor_tensor(
                    z[:], psq[:], BCt[:, tcc, 16 + n : 17 + n], en[:],
                    OP.mult, OP.mult,
                )
                accs.append(z)
                while len(accs) >= 2 and (
                    len(accs) % 2 == 0 or n == D_STATE - 1
                ):
                    b2 = accs.pop()
                    b1 = accs.pop()
                    zs = zp2.tile([128, 512], BF16, tag=f"acc{len(accs)}")
                    nc.vector.tensor_tensor(zs[:], b1[:], b2[:], OP.add)
                    accs.append(zs)
                    if len(accs) % 2 != 0 and n != D_STATE - 1:
                        break
            yfin = accs[0]
            if tcc == 0:
                # carry: h at t=127 for each state, rescaled to chunk start
                nc.scalar.activation(hend_raw[:], psend[:], AF.Copy)
                S127row = lcl.tile([1, 512], F32, tag="S127row")
                nc.gpsimd.dma_start(out=S127row[:], in_=S_sb[127:128, 0, :])
                S127b = lcl.tile([16, 512], F32, tag="S127b")
                nc.gpsimd.partition_broadcast(S127b[:], S127row[0:1, :])
                en_end = lcl.tile([16, 512], BF16, tag="en_end")
                nc.scalar.activation(
                    en_end[:], S127b[:], AF.Exp, scale=wt["nneg"][:]
                )
                nc.vector.tensor_tensor(hend[:], hend_raw[:], en_end[:], OP.mult)
            # y2 = y + D*u2 ; y3 = y2 * zsil2
            t1 = lc2.tile([128, 512], BF16, tag="t1")
            nc.vector.tensor_tensor(t1[:], u2T[:, tcc, :], dbc_t[:], OP.mult)
            y2 = lc2.tile([128, 512], BF16, tag="y2")
            nc.vector.tensor_tensor(y2[:], yfin[:], t1[:], OP.add)
            nc.vector.tensor_tensor(
                y3T[:, tcc, :], y2[:], zsil2T[:, tcc, :], OP.mult
            )

        # --- transpose y3 back to feature-major ---
        y3C = lcl.tile([128, 2, 4, 128], BF16, tag="y3C")
        for tcc in range(2):
            pst = tps.tile([128, 512], BF16, tag="tpb")
            for es in range(NES):
                nc.tensor.transpose(
                    pst[:, es * 128 : (es + 1) * 128],
                    y3T[:, tcc, es * 128 : (es + 1) * 128],
                    wt["eye"][:],
                )
            nc.scalar.activation(y3C[:, tcc, :, :], pst[:], AF.Copy)

        # --- out_proj (0.5 folded host-side) + residual ---
        for md in range(NMD):
            ps = mps.tile([128, 256], F32, tag="mm")
            for es in range(NES):
                nc.tensor.matmul(
                    ps[:],
                    wb[:, 2048 + (es * NMD + md) * 128 : 2048 + (es * NMD + md + 1) * 128],
                    y3C[:, :, es, :],
                    start=(es == 0),
                    stop=(es == NES - 1),
                )
            nc.vector.tensor_tensor(
                hresC[:, md, :], hresC[:, md, :], ps[:], OP.add
            )

    # ---------------- head ----------------
    sqC = lcl.tile([128, 2, 256], BF16, tag="sqC")
    for md in range(NMD):
        nc.scalar.activation(sqC[:, md, :], hresC[:, md, :], AF.Square)
    shrh = sps.tile([64, 256], F32, tag="shr")
    ssps = shrh[0:1, :]
    for md in range(NMD):
        nc.tensor.matmul(
            ssps, wt["ones"][:], sqC[:, md, :], start=(md == 0), stop=(md == NMD - 1)
        )
    eps1 = lcl.tile([1, 1], F32, tag="eps1")
    nc.vector.memset(eps1[:], 1e-5)
    sv = lcl.tile([1, 256], F32, tag="sv")
    nc.scalar.activation(sv[:], ssps, AF.Sqrt, scale=1.0 / 256.0, bias=eps1[:])
    rstd = lcl.tile([1, 256], F32, tag="rstd")
    nc.vector.reciprocal_approx_fast(rstd[:], sv[:])
    rb = lcl.tile([128, 256], F32, tag="rb")
    nc.gpsimd.partition_broadcast(rb[:], rstd[0:1, :])
    hnC = lcl.tile([128, 2, 256], BF16, tag="hnC")
    for md in range(NMD):
        nc.vector.tensor_tensor(hnC[:, md, :], hresC[:, md, :], rb[:], OP.mult)

    h1ps = sps.tile([64, 256], F32, tag="shr")
    for md in range(NMD):
        nc.tensor.matmul(
            h1ps[:], wt["hw1"][:, md * 64 : (md + 1) * 64], hnC[:, md, :],
            start=(md == 0), stop=(md == NMD - 1),
        )
    hhx = lcl.tile([64, 256], F32, tag="hhx")
    nc.scalar.activation(hhx[:], h1ps[:], AF.Identity, bias=wt["hb1"][:])
    hsq = lcl.tile([64, 256], F32, tag="hsq")
    nc.scalar.activation(hsq[:], hhx[:], AF.Square)
    hcu = lcl.tile([64, 256], F32, tag="hcu")
    nc.vector.tensor_tensor(hcu[:], hsq[:], hhx[:], OP.mult)
    harg = lcl.tile([64, 256], F32, tag="harg")
    nc.vector.scalar_tensor_tensor(
        harg[:], hcu[:], 0.044715, hhx[:], OP.mult, OP.add
    )
    hth = lcl.tile([64, 256], F32, tag="hth")
    nc.scalar.activation(hth[:], harg[:], AF.Tanh, scale=0.7978845608028654)
    hh = lcl.tile([64, 256], BF16, tag="hh")
    nc.vector.scalar_tensor_tensor(hh[:], hth[:], 1.0, hhx[:], OP.add, OP.mult)

    lgf = sps.tile([64, 256], F32, tag="shr")
    lgps = lgf[0:1, :]
    nc.tensor.matmul(lgps, wt["hw2"][:], hh[:], start=True, stop=True)
    lg = lcl.tile([1, 256], F32, tag="lgs")
    nc.scalar.activation(lg[:], lgps, AF.Identity, bias=wt["hb2"][0:1, 0:1])

    mx = lcl.tile([1, 1], F32, tag="mx")
    nc.vector.tensor_reduce(mx[:], lg[:], AX.X, OP.max)
    nm = lcl.tile([1, 1], F32, tag="nm")
    nc.vector.tensor_scalar_mul(nm[:], mx[:], -1.0)
    ex = lcl.tile([1, 256], F32, tag="ex")
    sm = lcl.tile([1, 1], F32, tag="sm")
    nc.scalar.activation(ex[:], lg[:], AF.Exp, bias=nm[:], accum_out=sm[:])
    rc = lcl.tile([1, 1], F32, tag="rc")
    nc.vector.reciprocal_approx_fast(rc[:], sm[:])
    wrow = lcl.tile([1, 256], F32, tag="wrow")
    nc.vector.tensor_scalar_mul(wrow[:], ex[:], rc[:])
    nc.vector.memset(wrow[:, 0:1], 0.0)
    nc.sync.dma_start(out=out_ap[:], in_=wrow[:])


# ---------------------------------------------------------------------------
# build + run
# ---------------------------------------------------------------------------

_CACHE = {}


def _build():
    if "nc" in _CACHE:
        return _CACHE["nc"]
    nc = bacc.Bacc("TRN2", target_bir_lowering=False, debug=False, num_devices=B)
    ins = {}
    ins["x"] = nc.dram_tensor("x", [3, H, T, W // 2], BF16, kind="ExternalInput").ap()
    for name, shape, dt in WSPECS:
        ins[name] = nc.dram_tensor(name, list(shape), dt, kind="ExternalInput").ap()
    out_ap = nc.dram_tensor("out", [1, T], F32, kind="ExternalOutput").ap()

    with tile.TileContext(nc) as tc:
        with ExitStack() as ctx:
            _emit(ctx, tc, ins, out_ap)
    nc.compile()
    _CACHE["nc"] = nc
    return nc


def _prep_x(x):
    """x: (B,T,1,H,W) f32 -> (B,3,H,T,W//2) bf16 shifted stride-2 copies."""
    xt = np.ascontiguousarray(np.transpose(x[:, :, 0], (0, 2, 1, 3)))  # (B,H,T,W)
    xsh = np.zeros((B, 3, H, T, W // 2), np.float32)
    ox = np.arange(W // 2)
    for kx in range(3):
        cols = 2 * ox + kx - 1
        v = (cols >= 0) & (cols < W)
        xsh[:, kx, :, :, v] = np.transpose(xt[:, :, :, cols[v]], (3, 0, 1, 2))
    return xsh.astype(BF)


def kernel(**inputs):
    wd = _host_prep(inputs)
    nc = _build()
    xsh = _prep_x(np.asarray(inputs["x"], np.float32))
    in_maps = []
    for b in range(B):
        m = dict(wd)
        m["x"] = np.ascontiguousarray(xsh[b])
        in_maps.append(m)
    res = run_bass_kernel_spmd(nc, in_maps, core_ids=list(range(B)))
    out = np.stack([res.results[b]["out"].reshape(T, 1) for b in range(B)])
    return out.astype(np.float32)


if __name__ == "__main__":
    import reference

    inp = {k: np.asarray(v) for k, v in reference.setup_inputs().items()}
    got = kernel(**inp)
    exp = np.asarray(reference.reference(**reference.setup_inputs()))
    err = np.abs(got - exp).max() / np.abs(exp).max()
    print("Relative error:", err)



# revision 24
# speedup vs baseline: 2.4243x; 2.4243x over previous
"""Trainium2 Bass kernel for nn_DYCEP_8572754723266.

CNN(3x stride-2 conv) -> fc -> 6x Mamba blocks -> head -> softmax-over-T.
Sharding: data-parallel over batch B=8, one batch element per NeuronCore.
"""

import numpy as np
import ml_dtypes
from contextlib import ExitStack

import concourse.bass as bass
import concourse.mybir as mybir
import concourse.tile as tile
from concourse import bacc
from concourse.bass_utils import run_bass_kernel_spmd

F32 = mybir.dt.float32
F32R = mybir.dt.float32r
BF16 = mybir.dt.bfloat16
AF = mybir.ActivationFunctionType
OP = mybir.AluOpType
AX = mybir.AxisListType

B, T, H, W = 8, 256, 64, 64
D_MODEL, N_LAYERS, D_STATE = 256, 6, 16
D_INNER = 2 * D_MODEL
DT_RANK = 16
D_CONV = 4
CNN_Z = 32
NES = 4  # d_inner slices of 128
NMD = 2  # d_model slices of 128

BF = ml_dtypes.bfloat16

# ---------------------------------------------------------------------------
# conv block tables
# ---------------------------------------------------------------------------


def _c1_blocks():
    # (lo, K): iy window per oy-block of 8; loaded into x-tile quadrant 32*b
    out = []
    for b in range(4):
        lo = max(0, 16 * b - 1)
        hi = min(63, 16 * b + 15)
        out.append((lo, hi - lo + 1))
    return out


def _c2_pieces():
    # z1 partition p = (iy % 8) * 16 + cin ; iyh = iy // 8 in free dim.
    # piece = ("full", iyh) K=128 base 0, or ("bnd", j) K=16 base 0 from z1b
    # (z1b row j holds iy = 8*j + 7).
    blocks = []
    for bp in range(4):
        pieces = []
        if bp > 0:
            pieces.append(("bnd", bp - 1))
        pieces.append(("full", bp))
        blocks.append(pieces)
    return blocks


def _c3_pieces():
    # z2 partition p = (iy % 4) * 32 + cin ; iyh = iy // 4.
    # piece = ("full", iyh) K=128 base 0, or ("bnd", iyh) K=32 base 96
    # (row iy = 4*iyh + 3 sits at partitions 96..127 of z2 block iyh).
    blocks = []
    for bp in range(4):
        pieces = []
        if bp > 0:
            pieces.append(("bnd", bp - 1))
        pieces.append(("full", bp))
        blocks.append(pieces)
    return blocks


C1B = _c1_blocks()
C2B = _c2_pieces()
C3B = _c3_pieces()


# padded coords: ix_pad = ix + 1, so matmul kx reads cols kx, kx+2, ... (full N)


# ---------------------------------------------------------------------------
# host-side weight preparation
# ---------------------------------------------------------------------------


def _host_prep(inp):
    d = {}
    f32 = np.float32

    w1 = np.asarray(inp["cnn_w1"], f32)
    w2 = np.asarray(inp["cnn_w2"], f32)
    w3 = np.asarray(inp["cnn_w3"], f32)

    # conv1 with kx folded into K: rows (kx, iy-rel), cols (block, oyl, cout)
    c1w = np.zeros((51, 4 * 128), f32)
    for b, (lo, K) in enumerate(C1B):
        for kx in range(3):
            for oyl in range(8):
                oy = 8 * b + oyl
                for cout in range(16):
                    m = oyl * 16 + cout
                    for r in range(K):
                        ky = (lo + r) - 2 * oy + 1
                        if 0 <= ky <= 2:
                            c1w[kx * K + r, b * 128 + m] = w1[cout, 0, ky, kx]
    d["c1w"] = c1w.astype(BF)

    n2 = sum(len(p) for p in C2B)
    c2w = np.zeros((128, n2 * 3 * 128), f32)
    idx = 0
    for bp, pieces in enumerate(C2B):
        for (kind, j) in pieces:
            rows = range(8 * j, 8 * j + 8) if kind == "full" else [8 * j + 7]
            for kx in range(3):
                col0 = idx * 128
                idx += 1
                for oyl in range(4):
                    oy = 4 * bp + oyl
                    for cout in range(32):
                        m = oyl * 32 + cout
                        for rr, iy in enumerate(rows):
                            ky = iy - 2 * oy + 1
                            if 0 <= ky <= 2:
                                c2w[rr * 16 : rr * 16 + 16, col0 + m] = w2[cout, :, ky, kx]
    d["c2w"] = c2w.astype(BF)

    n3 = sum(len(p) for p in C3B)
    c3w = np.zeros((128, n3 * 3 * 64), f32)
    idx = 0
    for bp, pieces in enumerate(C3B):
        for (kind, j) in pieces:
            if kind == "full":
                rows = [(rr, 4 * j + rr) for rr in range(4)]  # (slab row grp, iy)
                rbase = 0
            else:
                rows = [(0, 4 * j + 3)]
                rbase = 0
            for kx in range(3):
                col0 = idx * 64
                idx += 1
                for oyl in range(2):
                    oy = 2 * bp + oyl
                    for cout in range(32):
                        m = oyl * 32 + cout
                        for rr, iy in rows:
                            ky = iy - 2 * oy + 1
                            if 0 <= ky <= 2:
                                c3w[rbase + rr * 32 : rbase + rr * 32 + 32, col0 + m] = w3[
                                    cout, :, ky, kx
                                ]
    d["c3w"] = c3w.astype(BF)

    d["c1b"] = np.tile(np.asarray(inp["cnn_b1"], f32), 8).reshape(128, 1)
    d["c2b"] = np.tile(np.asarray(inp["cnn_b2"], f32), 4).reshape(128, 1)
    d["c3b"] = np.tile(np.asarray(inp["cnn_b3"], f32), 2).reshape(64, 1)

    fcw = np.asarray(inp["fc_w"], f32) / 64.0  # pool-mean folded
    d["fcw"] = np.ascontiguousarray(fcw.T).astype(BF)  # [32, 256]
    d["fcb"] = np.ascontiguousarray(
        np.asarray(inp["fc_b"], f32).reshape(NMD, 128).T
    )  # [128, 2]

    d["ones"] = np.ones((128, 1), f32).astype(BF)
    d["onesrow"] = np.ones((1, 256), f32).astype(BF)

    nw = np.asarray(inp["norm_w"], f32)
    ipw = np.asarray(inp["in_proj_w"], f32)
    xpw = np.asarray(inp["x_proj_w"], f32)
    dpw = np.asarray(inp["dt_proj_w"], f32)
    opw = np.asarray(inp["out_proj_w"], f32)
    cdw = np.asarray(inp["conv1d_w"], f32)
    cdb = np.asarray(inp["conv1d_b"], f32)
    dpb = np.asarray(inp["dt_proj_b"], f32)
    Dp = np.asarray(inp["Dp"], f32)

    # u2 = 2*silu(u-path), zsil2 = 2*silu(z): fold 0.5s into downstream weights.
    wbf = np.zeros((N_LAYERS, 128, 2048 + 1024 + 256 + 2048), f32)
    for l in range(N_LAYERS):
        wtl = (ipw[l] * nw[l][None, :]).T  # (256, 1024)
        # x-half as lhsT tiles (es 0..3), z-half as rhs [d, 512] per kd
        for kd in range(2):
            for es in range(NES):
                wbf[l, :, (kd * 4 + es) * 128 : (kd * 4 + es + 1) * 128] = wtl[
                    kd * 128 : (kd + 1) * 128, es * 128 : (es + 1) * 128
                ]
            wbf[l, :, 1024 + kd * 512 : 1024 + (kd + 1) * 512] = wtl[
                kd * 128 : (kd + 1) * 128, 512:1024
            ]
        otl = 0.5 * opw[l].T  # (512, 256); 0.5 from zsil2
        for es in range(NES):
            for md in range(NMD):
                wbf[l, :, 2048 + (es * NMD + md) * 128 : 2048 + (es * NMD + md + 1) * 128] = otl[
                    es * 128 : (es + 1) * 128, md * 128 : (md + 1) * 128
                ]
        # x_proj: 0.5 from u2; B rows get another 0.5 (du = delta*u2 = 2*delta*u)
        # output rows padded to 64: dt 0:16, pad, B 32:48, C 48:64
        xtl = 0.5 * xpw[l].T.copy()  # (512, 48)
        xtl[:, 16:32] *= 0.5
        xtl64 = np.zeros((512, 64), f32)
        xtl64[:, 0:16] = xtl[:, 0:16]
        xtl64[:, 32:64] = xtl[:, 16:48]
        for es in range(NES):
            wbf[l, :, 3072 + es * 64 : 3072 + (es + 1) * 64] = xtl64[
                es * 128 : (es + 1) * 128
            ]
        # depthwise conv1d as diagonal matmuls: lhsT = diag(w[es-slice, k])
        for es in range(NES):
            for k in range(4):
                col0 = 3328 + (es * 4 + k) * 128
                wbf[l, np.arange(128), col0 + np.arange(128)] = cdw[
                    l, es * 128 : (es + 1) * 128, k
                ]
    d["wbf"] = wbf.astype(BF)

    # dt_proj as rhs [17, 512]: rows 0:16 = dpw.T, row 16 = dpb (bias via
    # the augmented ones row of bcq2)
    dpt = np.empty((N_LAYERS, 17, 512), f32)
    for l in range(N_LAYERS):
        dpt[l, 0:16] = dpw[l].T
        dpt[l, 16] = dpb[l]
    d["dpw"] = dpt.astype(BF)

    # SSM helper constants
    tt_i = np.arange(128)
    d["tri"] = (tt_i[:, None] <= tt_i[None, :]).astype(BF)  # [t, t'] inclusive
    d["eye"] = np.eye(128, dtype=np.float32).astype(BF)
    eyeb = np.zeros((64, 32), np.float32)
    eyeb[32:64] = np.eye(32)
    d["eyeb"] = eyeb.astype(BF)
    sel = np.zeros((16, 16 * 128), f32)
    for n in range(16):
        sel[n, n * 128 : (n + 1) * 128] = 1.0
    d["sel"] = sel.astype(BF)
    d["nneg"] = -np.arange(1.0, 17.0, dtype=f32).reshape(16, 1)
    selc = np.zeros((128, 256), f32)
    for n in range(16):
        selc[:, n * 16 + n] = 1.0
    d["selc"] = selc.astype(BF)
    d["drow"] = (0.5 * Dp).astype(BF)  # [N_LAYERS, 512]

    # f32 pack: cdb (4) | cdb/2 (4) | dpb (4) | 0.5*Dp (4)
    wf = np.zeros((N_LAYERS, 128, 16), f32)
    wf[:, :, 0:4] = cdb.reshape(N_LAYERS, NES, 128).transpose(0, 2, 1)
    wf[:, :, 4:8] = 0.5 * cdb.reshape(N_LAYERS, NES, 128).transpose(0, 2, 1)
    wf[:, :, 8:12] = dpb.reshape(N_LAYERS, NES, 128).transpose(0, 2, 1)
    wf[:, :, 12:16] = 0.5 * Dp.reshape(N_LAYERS, NES, 128).transpose(0, 2, 1)
    d["wf32"] = wf

    nfw = np.asarray(inp["norm_f_w"], f32)
    hw1 = np.asarray(inp["head_w1"], f32) * nfw[None, :]
    hw1t = hw1.T  # (256, 64)
    d["hw1"] = np.concatenate([hw1t[0:128], hw1t[128:256]], axis=1).astype(BF)
    d["hb1"] = np.asarray(inp["head_b1"], f32).reshape(64, 1)
    d["hw2"] = np.ascontiguousarray(0.5 * np.asarray(inp["head_w2"], f32).T).astype(BF)
    d["hb2"] = np.asarray(inp["head_b2"], f32).reshape(1, 1)
    return d


WSPECS = [
    ("c1w", (51, 4 * 128), BF16),
    ("c2w", (128, sum(len(p) for p in C2B) * 3 * 128), BF16),
    ("c3w", (128, sum(len(p) for p in C3B) * 3 * 64), BF16),
    ("c1b", (128, 1), F32),
    ("c2b", (128, 1), F32),
    ("c3b", (64, 1), F32),
    ("fcw", (32, 256), BF16),
    ("fcb", (128, 2), F32),
    ("ones", (128, 1), BF16),
    ("onesrow", (1, 256), BF16),
    ("wbf", (N_LAYERS, 128, 2048 + 1024 + 256 + 2048), BF16),
    ("dpw", (N_LAYERS, 17, 512), BF16),
    ("wf32", (N_LAYERS, 128, 16), F32),
    ("hw1", (128, 128), BF16),
    ("hb1", (64, 1), F32),
    ("hw2", (64, 1), BF16),
    ("hb2", (1, 1), F32),
    ("tri", (128, 128), BF16),
    ("eye", (128, 128), BF16),
    ("eyeb", (64, 32), BF16),
    ("sel", (16, 16 * 128), BF16),
    ("selc", (128, 256), BF16),
    ("nneg", (16, 1), F32),
    ("drow", (N_LAYERS, 512), BF16),
]


# ---------------------------------------------------------------------------
# device program
# ---------------------------------------------------------------------------


def _emit(ctx: ExitStack, tc, ins, out_ap):
    nc = tc.nc
    x = ins["x"]

    wsb = ctx.enter_context(tc.tile_pool(name="wsb", bufs=1))
    wt = {}
    for name in ("c1w", "c2w", "c3w", "c1b", "c2b", "c3b", "fcw", "fcb", "ones",
                 "hw1", "hb1", "hw2", "hb2", "tri", "eye", "eyeb", "sel",
                 "selc", "nneg"):
        ap = ins[name]
        t = wsb.tile(list(ap.shape), ap.dtype, tag=name)
        nc.sync.dma_start(out=t[:], in_=ap[:])
        wt[name] = t

    hp = ctx.enter_context(tc.tile_pool(name="hres", bufs=1))
    hresC = hp.tile([128, 2, 256], F32, tag="hresC")
    zpp = ctx.enter_context(tc.tile_pool(name="zpp", bufs=1))

    # ---------------- CNN ----------------
    with ExitStack() as cnx:
        xp = cnx.enter_context(tc.tile_pool(name="xp", bufs=3))
        z1p = cnx.enter_context(tc.tile_pool(name="z1p", bufs=2))
        z2p = cnx.enter_context(tc.tile_pool(name="z2p", bufs=2))
        z3p = cnx.enter_context(tc.tile_pool(name="z3p", bufs=2))
        cp1 = cnx.enter_context(tc.tile_pool(name="cp1", bufs=4, space="PSUM"))
        cp2 = cnx.enter_context(tc.tile_pool(name="cp2", bufs=2, space="PSUM"))
        cp3 = cnx.enter_context(tc.tile_pool(name="cp3", bufs=2, space="PSUM"))

        zp = zpp.tile([64, 256], F32)

        for c64 in range(4):
            z3 = z3p.tile([64, 64, 4, 8], BF16)  # (f64, oyh, ox)
            z2 = z2p.tile([128, 2, 32, 4, 18], BF16)  # (c32, f32, iyh, ixpad)
            z2b = z2p.tile([32, 2, 32, 3, 18], BF16, tag="z2b")  # bnd rows
            nc.vector.memset(z2[:, :, :, :, 0:1], 0.0)
            nc.vector.memset(z2[:, :, :, :, 17:18], 0.0)
            for c32 in range(2):
                z1 = z1p.tile([128, 2, 16, 4, 34], BF16)  # (c16, f16, iyh, ixpad)
                nc.vector.memset(z1[:, :, :, :, 0:1], 0.0)
                nc.vector.memset(z1[:, :, :, :, 33:34], 0.0)
                z1b = z1p.tile([16, 2, 16, 3, 34], BF16, tag="z1b")  # bnd rows
                for c16 in range(2):
                    f0 = (c64 * 4 + c32 * 2 + c16) * 16
                    for b, (lo, K) in enumerate(C1B):
                        xk = xp.tile([51, 16, 32], BF16, tag=f"xk{b}")
                        for kx in range(3):
                            nc.sync.dma_start(
                                out=xk[kx * K : (kx + 1) * K],
                                in_=x[kx, lo : lo + K, f0 : f0 + 16, :],
                            )
                        ps = cp1.tile([128, 16, 32], F32)
                        nc.tensor.matmul(
                            ps[:],
                            wt["c1w"][0 : 3 * K, b * 128 : (b + 1) * 128],
                            xk[0 : 3 * K],
                            start=True,
                            stop=True,
                        )
                        nc.scalar.activation(
                            z1[:, c16, :, b, 1:33], ps[:], AF.Relu, bias=wt["c1b"][:]
                        )
                        if b < 3:
                            nc.gpsimd.dma_start(
                                out=z1b[:, c16, :, b, :],
                                in_=z1[112:128, c16, :, b, :],
                            )
                # conv2 over the 32-frame chunk
                for bp, pieces in enumerate(C2B):
                    ps = cp2.tile([128, 32, 16], F32)
                    nmm = len(pieces) * 3
                    im = 0
                    for pi, (kind, j) in enumerate(pieces):
                        pidx = sum(len(p) for p in C2B[:bp]) + pi
                        for kx in range(3):
                            if kind == "full":
                                rhs = z1[:, :, :, j, kx : kx + 31 : 2]
                                K = 128
                            else:
                                rhs = z1b[:, :, :, j, kx : kx + 31 : 2]
                                K = 16
                            lhs = wt["c2w"][
                                0:K,
                                (pidx * 3 + kx) * 128 : (pidx * 3 + kx + 1) * 128,
                            ]
                            im += 1
                            nc.tensor.matmul(
                                ps[:],
                                lhs,
                                rhs,
                                start=(im == 1),
                                stop=(im == nmm),
                            )
                    nc.vector.tensor_scalar(
                        z2[:, c32, :, bp, 1:17], ps[:], wt["c2b"][:], 0.0,
                        OP.add, OP.max,
                    )
                    if bp < 3:
                        nc.gpsimd.dma_start(
                            out=z2b[:, c32, :, bp, :],
                            in_=z2[96:128, c32, :, bp, :],
                        )
            # conv3 over the 64-frame chunk
            for bp, pieces in enumerate(C3B):
                ps = cp3.tile([64, 64, 8], F32)
                nmm = len(pieces) * 3
                im = 0
                for pi, (kind, j) in enumerate(pieces):
                    pidx = sum(len(p) for p in C3B[:bp]) + pi
                    for kx in range(3):
                        if kind == "full":
                            rhs = z2[:, :, :, j, kx : kx + 15 : 2]
                            lhs = wt["c3w"][
                                0:128,
                                (pidx * 3 + kx) * 64 : (pidx * 3 + kx + 1) * 64,
                            ]
                        else:
                            rhs = z2b[:, :, :, j, kx : kx + 15 : 2]
                            lhs = wt["c3w"][
                                0:32,
                                (pidx * 3 + kx) * 64 : (pidx * 3 + kx + 1) * 64,
                            ]
                        im += 1
                        nc.tensor.matmul(
                            ps[:],
                            lhs,
                            rhs,
                            start=(im == 1),
                            stop=(im == nmm),
                        )
                nc.scalar.activation(
                    z3[:, :, bp, :], ps[:], AF.Relu, bias=wt["c3b"][:]
                )
            # spatial mean (x 1/64 folded into fcw): sum over (oyh, ox)
            nc.vector.tensor_reduce(
                zp[:, c64 * 64 : (c64 + 1) * 64], z3[:], AX.XY, OP.add
            )

        # fold (oyl 2) partition pairs: zq = zp[0:32] + zp[32:64]
        zq = zpp.tile([32, 256], F32, tag="zq")
        nc.sync.dma_start(out=zq[:], in_=zp[32:64, :])
        zfold = zpp.tile([32, 256], BF16, tag="zfold")
        nc.vector.tensor_tensor(zfold[:], zp[0:32, :], zq[:], OP.add)

    # ---------------- fc (CNN pools closed; use mamba psum pool) ----------------
    lwp = ctx.enter_context(tc.tile_pool(name="lwp", bufs=2))
    mps = ctx.enter_context(tc.tile_pool(name="mps", bufs=2, space="PSUM"))
    sps = ctx.enter_context(tc.tile_pool(name="sps", bufs=1, space="PSUM"))
    bps = ctx.enter_context(tc.tile_pool(name="bps", bufs=1, space="PSUM"))
    tps = ctx.enter_context(tc.tile_pool(name="tps", bufs=1, space="PSUM"))
    qps = ctx.enter_context(tc.tile_pool(name="qps", bufs=2, space="PSUM"))
    lcl = ctx.enter_context(tc.tile_pool(name="lcl", bufs=1))
    lc2 = ctx.enter_context(tc.tile_pool(name="lc2", bufs=2))
    epp = ctx.enter_context(tc.tile_pool(name="epp", bufs=3))
    zp2 = ctx.enter_context(tc.tile_pool(name="zp2", bufs=3))

    for md in range(NMD):
        ps = mps.tile([128, 256], F32, tag="mm")
        nc.tensor.matmul(
            ps[:], wt["fcw"][:, md * 128 : (md + 1) * 128], zfold[:],
            start=True, stop=True,
        )
        nc.scalar.activation(
            hresC[:, md, :], ps[:], AF.Identity, bias=wt["fcb"][:, md : md + 1]
        )

    # ---------------- Mamba layers (time-major SSM) ----------------
    for l in range(N_LAYERS):
        wb = lwp.tile([128, 5376], BF16, tag="wb")
        nc.gpsimd.dma_start(out=wb[:], in_=ins["wbf"][l])
        dpw_t = lwp.tile([17, 512], BF16, tag="dpw")
        nc.gpsimd.dma_start(out=dpw_t[:], in_=ins["dpw"][l])
        wf = lwp.tile([128, 16], F32, tag="wf")
        nc.gpsimd.dma_start(out=wf[:], in_=ins["wf32"][l])
        drow = lwp.tile([1, 512], BF16, tag="drow")
        nc.gpsimd.dma_start(out=drow[:], in_=ins["drow"][l : l + 1, :])
        dbc_t = lwp.tile([128, 512], BF16, tag="dbc")
        nc.gpsimd.partition_broadcast(dbc_t[:], drow[0:1, :])

        # --- rmsnorm (norm_w folded into in_proj weights) ---
        sqC = lcl.tile([128, 2, 256], BF16, tag="sqC")
        for md in range(NMD):
            nc.scalar.activation(sqC[:, md, :], hresC[:, md, :], AF.Square)
        shr = sps.tile([64, 256], F32, tag="shr")
        ssps = shr[0:1, :]
        for md in range(NMD):
            nc.tensor.matmul(
                ssps, wt["ones"][:], sqC[:, md, :],
                start=(md == 0), stop=(md == NMD - 1),
            )
        eps1 = lcl.tile([1, 1], F32, tag="eps1")
        nc.vector.memset(eps1[:], 1e-5)
        sv = lcl.tile([1, 256], F32, tag="sv")
        nc.scalar.activation(sv[:], ssps, AF.Sqrt, scale=1.0 / 256.0, bias=eps1[:])
        rstd = lcl.tile([1, 256], F32, tag="rstd")
        nc.vector.reciprocal_approx_fast(rstd[:], sv[:])
        rb = lcl.tile([128, 256], F32, tag="rb")
        nc.gpsimd.partition_broadcast(rb[:], rstd[0:1, :])
        hnC = lcl.tile([128, 2, 256], BF16, tag="hnC")
        for md in range(NMD):
            nc.vector.tensor_tensor(hnC[:, md, :], hresC[:, md, :], rb[:], OP.mult)

        # --- in_proj x-half (feature-major, es 0..3) ---
        xinC = lcl.tile([128, 4, 260], BF16, tag="xinC")
        nc.vector.memset(xinC[:, :, 0:3], 0.0)
        for es in range(NES):
            ps = mps.tile([128, 256], F32, tag="mm")
            for kd in range(2):
                nc.tensor.matmul(
                    ps[:],
                    wb[:, (kd * 4 + es) * 128 : (kd * 4 + es + 1) * 128],
                    hnC[:, kd, :],
                    start=(kd == 0),
                    stop=(kd == 1),
                )
            nc.scalar.activation(xinC[:, es, 3:259], ps[:], AF.Copy)

        # --- in_proj z-half (time-major) + silu ---
        zsil2T = lcl.tile([128, 2, 512], BF16, tag="zsil2T")
        for tcc in range(2):
            psz = bps.tile([128, 512], F32, tag="big")
            for kd in range(2):
                nc.tensor.matmul(
                    psz[:],
                    hnC[:, kd, tcc * 128 : (tcc + 1) * 128],
                    wb[:, 1024 + kd * 512 : 1024 + (kd + 1) * 512],
                    start=(kd == 0),
                    stop=(kd == 1),
                )
            thz = lc2.tile([128, 512], BF16, tag="thz")
            zc = lc2.tile([128, 512], BF16, tag="zc")
            nc.scalar.activation(thz[:], psz[:], AF.Tanh, scale=0.5)
            nc.scalar.activation(zc[:], psz[:], AF.Copy)
            nc.vector.scalar_tensor_tensor(
                zsil2T[:, tcc, :], thz[:], 1.0, zc[:], OP.add, OP.mult
            )

        # --- depthwise conv1d as diagonal PE matmuls (feature-major) ---
        xcC = lcl.tile([128, 4, 256], BF16, tag="xcC")
        thuC = lcl.tile([128, 4, 256], BF16, tag="thuC")
        for es in range(NES):
            pc = mps.tile([128, 256], F32, tag="mm")
            for k in range(4):
                nc.tensor.matmul(
                    pc[:],
                    wb[:, 3328 + (es * 4 + k) * 128 : 3328 + (es * 4 + k + 1) * 128],
                    xinC[:, es, k : k + 256],
                    start=(k == 0),
                    stop=(k == 3),
                )
            nc.scalar.activation(
                xcC[:, es, :], pc[:], AF.Identity, bias=wf[:, 0 + es : 1 + es]
            )
            nc.scalar.activation(
                thuC[:, es, :], pc[:], AF.Tanh, scale=0.5,
                bias=wf[:, 4 + es : 5 + es],
            )
        u2C = lcl.tile([128, 4, 256], BF16, tag="u2C")
        nc.vector.scalar_tensor_tensor(
            u2C[:].rearrange("p a t -> p (a t)"),
            thuC[:].rearrange("p a t -> p (a t)"),
            1.0,
            xcC[:].rearrange("p a t -> p (a t)"),
            OP.add,
            OP.mult,
        )

        # --- x_proj -> (dt, B, C); augmented with a ones row for dt bias ---
        shr2 = sps.tile([64, 256], F32, tag="shr")
        dbc = shr2[:]
        for es in range(NES):
            nc.tensor.matmul(
                dbc, wb[:, 3072 + es * 64 : 3072 + (es + 1) * 64], u2C[:, es, :],
                start=(es == 0), stop=(es == NES - 1),
            )
        bcq2 = lcl.tile([64, 256], BF16, tag="bcq2")
        nc.scalar.activation(bcq2[0:16, :], dbc[0:16, :], AF.Copy)
        nc.gpsimd.dma_start(out=bcq2[16:17, :], in_=ins["onesrow"][:])
        nc.scalar.activation(bcq2[32:64, :], dbc[32:64, :], AF.Copy)

        # --- B/C transpose to time-major per-partition scalars ---
        BCt = lcl.tile([128, 2, 32], F32, tag="BCt")
        for tcc in range(2):
            pstf = tps.tile([128, 512], BF16, tag="tpb")
            pst = pstf[:, 0:32]
            nc.tensor.transpose(
                pst, bcq2[32:64, tcc * 128 : (tcc + 1) * 128],
                wt["eyeb"][32:64, :],
            )
            nc.scalar.activation(BCt[:, tcc, :], pst, AF.Copy)

        # --- dt_proj (time-major) -> delta; softplus via 2-term series ---
        deltaT = lcl.tile([128, 2, 512], BF16, tag="deltaT")
        for tcc in range(2):
            psd = bps.tile([128, 512], F32, tag="big")
            nc.tensor.matmul(
                psd[:], bcq2[0:17, tcc * 128 : (tcc + 1) * 128], dpw_t[:],
                start=True, stop=True,
            )
            spe = lc2.tile([128, 512], F32, tag="spe")
            nc.scalar.activation(spe[:], psd[:], AF.Exp)
            sp1 = lc2.tile([128, 512], F32, tag="sp1")
            nc.vector.tensor_scalar(sp1[:], spe[:], -0.5, 1.0, OP.mult, OP.add)
            nc.vector.tensor_tensor(deltaT[:, tcc, :], spe[:], sp1[:], OP.mult)

        # --- u2 transpose (PE) -> u2T; du = delta * u2 ---
        u2T = lcl.tile([128, 2, 512], BF16, tag="u2T")
        duT = lcl.tile([128, 2, 512], BF16, tag="duT")
        for tcc in range(2):
            pst = tps.tile([128, 512], BF16, tag="tpb")
            for es in range(NES):
                nc.tensor.transpose(
                    pst[:, es * 128 : (es + 1) * 128],
                    u2C[:, es, tcc * 128 : (tcc + 1) * 128],
                    wt["eye"][:],
                )
            nc.scalar.activation(u2T[:, tcc, :], pst[:], AF.Copy)
            nc.vector.tensor_tensor(
                duT[:, tcc, :], deltaT[:, tcc, :], u2T[:, tcc, :], OP.mult
            )

        # --- S = inclusive cumsum of delta over t (PE triangular matmul) ---
        S_sb = lcl.tile([128, 2, 512], F32, tag="S_sb")
        for tcc in range(2):
            psS = bps.tile([128, 512], F32, tag="big")
            nc.tensor.matmul(
                psS[:], wt["tri"][:], deltaT[:, tcc, :], start=True, stop=True
            )
            nc.scalar.activation(S_sb[:, tcc, :], psS[:], AF.Copy)

        # --- SSM: y[t,e] = sum_n C_n[t] * en_n * cumsum_t(du*B_n*ep_n) ---
        hend_raw = lcl.tile([16, 512], F32, tag="hend_raw")
        hend = lcl.tile([16, 512], BF16, tag="hend")
        y3T = lcl.tile([128, 2, 512], BF16, tag="y3T")
        for tcc in range(2):
            accs = []
            if tcc == 0:
                psend = sps.tile([16, 512], F32, tag="hendp")
            for n in range(D_STATE):
                a = float(n + 1)
                ep = epp.tile([128, 512], BF16, tag="ep")
                nc.scalar.activation(ep[:], S_sb[:, tcc, :], AF.Exp, scale=a)
                en = epp.tile([128, 512], BF16, tag="en")
                nc.scalar.activation(en[:], S_sb[:, tcc, :], AF.Exp, scale=-a)
                q = epp.tile([128, 512], BF16, tag="q")
                nc.vector.scalar_tensor_tensor(
                    q[:], duT[:, tcc, :], BCt[:, tcc, n : n + 1], ep[:],
                    OP.mult, OP.mult,
                )
                psq = qps.tile([128, 512], F32, tag="qs")
                nc.tensor.matmul(
                    psq[:], wt["tri"][:], q[:], start=True, stop=(tcc == 0)
                )
                if tcc == 1:
                    nc.tensor.matmul(
                        psq[:], wt["sel"][:, n * 128 : (n + 1) * 128], hend[:],
                        start=False, stop=True,
                    )
                if tcc == 0:
                    # route sum_t q_n[t,e] to partition n of psend
                    nc.tensor.matmul(
                        psend[:], wt["selc"][:, n * 16 : (n + 1) * 16], q[:],
                        start=(n == 0), stop=(n == D_STATE - 1),
                    )
                z = zp2.tile([128, 512], BF16, tag=f"z{n % 4}")
                nc.vector.scalar_tensor_tensor(
                    z[:], psq[:], BCt[:, tcc, 16 + n : 17 + n], en[:],
                    OP.mult, OP.mult,
                )
                accs.append(z)
                while len(accs) >= 2 and (
                    len(accs) % 2 == 0 or n == D_STATE - 1
                ):
                    b2 = accs.pop()
                    b1 = accs.pop()
                    zs = zp2.tile([128, 512], BF16, tag=f"acc{len(accs)}")
                    nc.vector.tensor_tensor(zs[:], b1[:], b2[:], OP.add)
                    accs.append(zs)
                    if len(accs) % 2 != 0 and n != D_STATE - 1:
                        break
            yfin = accs[0]
            if tcc == 0:
                # carry: h at t=127 for each state, rescaled to chunk start
                nc.scalar.activation(hend_raw[:], psend[:], AF.Copy)
                S127row = lcl.tile([1, 512], F32, tag="S127row")
                nc.gpsimd.dma_start(out=S127row[:], in_=S_sb[127:128, 0, :])
                S127b = lcl.tile([16, 512], F32, tag="S127b")
                nc.gpsimd.partition_broadcast(S127b[:], S127row[0:1, :])
                en_end = lcl.tile([16, 512], BF16, tag="en_end")
                nc.scalar.activation(
                    en_end[:], S127b[:], AF.Exp, scale=wt["nneg"][:]
                )
                nc.vector.tensor_tensor(hend[:], hend_raw[:], en_end[:], OP.mult)
            # y2 = y + D*u2 ; y3 = y2 * zsil2
            t1 = lc2.tile([128, 512], BF16, tag="t1")
            nc.vector.tensor_tensor(t1[:], u2T[:, tcc, :], dbc_t[:], OP.mult)
            y2 = lc2.tile([128, 512], BF16, tag="y2")
            nc.vector.tensor_tensor(y2[:], yfin[:], t1[:], OP.add)
            nc.vector.tensor_tensor(
                y3T[:, tcc, :], y2[:], zsil2T[:, tcc, :], OP.mult
            )

        # --- transpose y3 back to feature-major ---
        y3C = lcl.tile([128, 2, 4, 128], BF16, tag="y3C")
        for tcc in range(2):
            pst = tps.tile([128, 512], BF16, tag="tpb")
            for es in range(NES):
                nc.tensor.transpose(
                    pst[:, es * 128 : (es + 1) * 128],
                    y3T[:, tcc, es * 128 : (es + 1) * 128],
                    wt["eye"][:],
                )
            nc.scalar.activation(y3C[:, tcc, :, :], pst[:], AF.Copy)

        # --- out_proj (0.5 folded host-side) + residual ---
        for md in range(NMD):
            ps = mps.tile([128, 256], F32, tag="mm")
            for es in range(NES):
                nc.tensor.matmul(
                    ps[:],
                    wb[:, 2048 + (es * NMD + md) * 128 : 2048 + (es * NMD + md + 1) * 128],
                    y3C[:, :, es, :],
                    start=(es == 0),
                    stop=(es == NES - 1),
                )
            nc.vector.tensor_tensor(
                hresC[:, md, :], hresC[:, md, :], ps[:], OP.add
            )

    # ---------------- head ----------------
    sqC = lcl.tile([128, 2, 256], BF16, tag="sqC")
    for md in range(NMD):
        nc.scalar.activation(sqC[:, md, :], hresC[:, md, :], AF.Square)
    shrh = sps.tile([64, 256], F32, tag="shr")
    ssps = shrh[0:1, :]
    for md in range(NMD):
        nc.tensor.matmul(
            ssps, wt["ones"][:], sqC[:, md, :], start=(md == 0), stop=(md == NMD - 1)
        )
    eps1 = lcl.tile([1, 1], F32, tag="eps1")
    nc.vector.memset(eps1[:], 1e-5)
    sv = lcl.tile([1, 256], F32, tag="sv")
    nc.scalar.activation(sv[:], ssps, AF.Sqrt, scale=1.0 / 256.0, bias=eps1[:])
    rstd = lcl.tile([1, 256], F32, tag="rstd")
    nc.vector.reciprocal_approx_fast(rstd[:], sv[:])
    rb = lcl.tile([128, 256], F32, tag="rb")
    nc.gpsimd.partition_broadcast(rb[:], rstd[0:1, :])
    hnC = lcl.tile([128, 2, 256], BF16, tag="hnC")
    for md in range(NMD):
        nc.vector.tensor_tensor(hnC[:, md, :], hresC[:, md, :], rb[:], OP.mult)

    h1ps = sps.tile([64, 256], F32, tag="shr")
    for md in range(NMD):
        nc.tensor.matmul(
            h1ps[:], wt["hw1"][:, md * 64 : (md + 1) * 64], hnC[:, md, :],
            start=(md == 0), stop=(md == NMD - 1),
        )
    hhx = lcl.tile([64, 256], F32, tag="hhx")
    nc.scalar.activation(hhx[:], h1ps[:], AF.Identity, bias=wt["hb1"][:])
    hsq = lcl.tile([64, 256], F32, tag="hsq")
    nc.scalar.activation(hsq[:], hhx[:], AF.Square)
    hcu = lcl.tile([64, 256], F32, tag="hcu")
    nc.vector.tensor_tensor(hcu[:], hsq[:], hhx[:], OP.mult)
    harg = lcl.tile([64, 256], F32, tag="harg")
    nc.vector.scalar_tensor_tensor(
        harg[:], hcu[:], 0.044715, hhx[:], OP.mult, OP.add
    )
    hth = lcl.tile([64, 256], F32, tag="hth")
    nc.scalar.activation(hth[:], harg[:], AF.Tanh, scale=0.7978845608028654)
    hh = lcl.tile([64, 256], BF16, tag="hh")
    nc.vector.scalar_tensor_tensor(hh[:], hth[:], 1.0, hhx[:], OP.add, OP.mult)

    lgf = sps.tile([64, 256], F32, tag="shr")
    lgps = lgf[0:1, :]
    nc.tensor.matmul(lgps, wt["hw2"][:], hh[:], start=True, stop=True)
    lg = lcl.tile([1, 256], F32, tag="lgs")
    nc.scalar.activation(lg[:], lgps, AF.Identity, bias=wt["hb2"][0:1, 0:1])

    mx = lcl.tile([1, 1], F32, tag="mx")
    nc.vector.tensor_reduce(mx[:], lg[:], AX.X, OP.max)
    nm = lcl.tile([1, 1], F32, tag="nm")
    nc.vector.tensor_scalar_mul(nm[:], mx[:], -1.0)
    ex = lcl.tile([1, 256], F32, tag="ex")
    sm = lcl.tile([1, 1], F32, tag="sm")
    nc.scalar.activation(ex[:], lg[:], AF.Exp, bias=nm[:], accum_out=sm[:])
    rc = lcl.tile([1, 1], F32, tag="rc")
    nc.vector.reciprocal_approx_fast(rc[:], sm[:])
    wrow = lcl.tile([1, 256], F32, tag="wrow")
    nc.vector.tensor_scalar_mul(wrow[:], ex[:], rc[:])
    nc.vector.memset(wrow[:, 0:1], 0.0)
    nc.sync.dma_start(out=out_ap[:], in_=wrow[:])


# ---------------------------------------------------------------------------
# build + run
# ---------------------------------------------------------------------------

_CACHE = {}


def _build():
    if "nc" in _CACHE:
        return _CACHE["nc"]
    nc = bacc.Bacc("TRN2", target_bir_lowering=False, debug=False, num_devices=B)
    ins = {}
    ins["x"] = nc.dram_tensor("x", [3, H, T, W // 2], BF16, kind="ExternalInput").ap()
    for name, shape, dt in WSPECS:
        ins[name] = nc.dram_tensor(name, list(shape), dt, kind="ExternalInput").ap()
    out_ap = nc.dram_tensor("out", [1, T], F32, kind="ExternalOutput").ap()

    with tile.TileContext(nc) as tc:
        with ExitStack() as ctx:
            _emit(ctx, tc, ins, out_ap)
    nc.compile()
    _CACHE["nc"] = nc
    return nc


def _prep_x(x):
    """x: (B,T,1,H,W) f32 -> (B,3,H,T,W//2) bf16 shifted stride-2 copies."""
    xt = np.ascontiguousarray(np.transpose(x[:, :, 0], (0, 2, 1, 3)))  # (B,H,T,W)
    xsh = np.zeros((B, 3, H, T, W // 2), np.float32)
    ox = np.arange(W // 2)
    for kx in range(3):
        cols = 2 * ox + kx - 1
        v = (cols >= 0) & (cols < W)
        xsh[:, kx, :, :, v] = np.transpose(xt[:, :, :, cols[v]], (3, 0, 1, 2))
    return xsh.astype(BF)


def kernel(**inputs):
    wd = _host_prep(inputs)
    nc = _build()
    xsh = _prep_x(np.asarray(inputs["x"], np.float32))
    in_maps = []
    for b in range(B):
        m = dict(wd)
        m["x"] = np.ascontiguousarray(xsh[b])
        in_maps.append(m)
    res = run_bass_kernel_spmd(nc, in_maps, core_ids=list(range(B)))
    out = np.stack([res.results[b]["out"].reshape(T, 1) for b in range(B)])
    return out.astype(np.float32)


if __name__ == "__main__":
    import reference

    inp = {k: np.asarray(v) for k, v in reference.setup_inputs().items()}
    got = kernel(**inp)
    exp = np.asarray(reference.reference(**reference.setup_inputs()))
    err = np.abs(got - exp).max() / np.abs(exp).max()
    print("Relative error:", err)

